# revision 6
# baseline (speedup 1.0000x reference)
"""DSAttention Trainium2 kernel (v4: 512-col q-windows, row-tiled QK pairs).

Reference computation (per batch b, head h):
    S[q,s]  = (Q[q]·K[s]) * tau[b] + delta[b,s]
    S      += causal mask (s > q -> -inf)
    A       = softmax(S / sqrt(E), axis=s)
    O[q,:]  = sum_s A[q,s] * V[s,:]

Shapes: B=2, L=2048, H=16, E=64 -> 32 (b,h) pairs, 4 per NeuronCore x 8 cores.

Host-side algebraic folds:
  - tau folds into K:  kt = (K * tau)^T                  (bf16, dup [128, L])
  - delta folds into V: A = exp((QK*tau)/8) * D_s with D_s = exp(delta_s/8);
    O = (V*D)^T A / (D^T A), so v1 = [V*D | D] ([128, NCH, 65] chunks).

Device design:
  - q is processed in 512-col windows (4 per head).  Per window, s-chunks
    n = 0..4w+3 each produce S^T scores in their own [128,512] PSUM tile
    (1 bank): 6 tiles in the pool, so the QK->exp->AV pipeline never
    stalls on slot recycling and window boundaries refill instantly.
  - QK contraction is only E=64, so Q^T/K^T are duplicated on SBUF
    partition halves and even/odd chunks run as CONCURRENT 64-row PE
    tiles (tile_position (0,0)/(64,0)) -> ~2x QK throughput.
  - exp is assigned per-chunk to one engine: DVE does a single
    scalar_tensor_tensor Schraudolph (fused causal mask via tribias,
    output bf16 bits as saturating int16) on diagonal chunks plus some
    interior ones; ScalarE does exact exp on the rest.  Whole-chunk
    assignment keeps the ScalarE instruction count (213ns each fixed
    cost) low.
  - AV accumulates [V*D | D]^T @ A^T into a [65,512] PSUM tile; the
    denominator rides in row 64.  One PSUM->SBUF copy per window
    (alternating DVE/ScalarE), then DMA out; host divides.
  - exp ACT table preloaded via a dummy activation; ~6 junk warmup MMs
    bridge the initial DMA so the PE HAM clock gate opens early.
"""

import math
import sys

sys.path.insert(0, "/opt/trn_rl_repo")

import ml_dtypes
import numpy as np

import concourse.bass as bass
import concourse.tile as tile
from concourse import bacc, mybir
from concourse.masks import make_upper_triangular

B, L, H, E = 2, 2048, 16, 64
NCORES = 8
HPC = (B * H) // NCORES  # heads per core = 4
NCH = L // 128  # s-chunks per head = 16
NWIN = L // 512  # q-windows per head = 4
SCALE = 1.0 / 8.0  # 1/sqrt(E)
F32 = mybir.dt.float32
I16 = mybir.dt.int16
BF16 = mybir.dt.bfloat16
EXP = mybir.ActivationFunctionType.Exp
COPYF = mybir.ActivationFunctionType.Copy
MULT = mybir.AluOpType.mult
ADD = mybir.AluOpType.add

# Schraudolph constants (bf16 bit trick): bits = round(ps*C1 + BIAS)
LOG2E = 1.4426950408889634
C1 = 128.0 * LOG2E * SCALE  # 16*log2(e)
C_ADJ = 7.0  # error-centering constant
BIAS = 127.0 * 128.0 - C_ADJ
BIG = 1.0e9  # pushes masked entries to int16 saturation = bf16 -0.0

WARMUP_MMS = 6
LAG = 5  # chunks of QK/exp emitted ahead of their AV

# Interior chunks whose exp runs whole on DVE (diag chunks are always DVE).
DVE_INT = {0: frozenset(), 1: frozenset({1}), 2: frozenset({1, 5}),
           3: frozenset({1, 5, 9})}


def _body(tc, qT, kT, v1, out):
    nc = tc.nc
    from contextlib import ExitStack

    with ExitStack() as ctx:
        const = ctx.enter_context(tc.tile_pool(name="const", bufs=1))
        qk_pool = ctx.enter_context(tc.tile_pool(name="qk", bufs=2))
        v_pool = ctx.enter_context(tc.tile_pool(name="v", bufs=2))
        a_pool = ctx.enter_context(tc.tile_pool(name="a", bufs=LAG + 2))
        o_pool = ctx.enter_context(tc.tile_pool(name="o", bufs=8))
        ps_pool = ctx.enter_context(tc.tile_pool(name="psS", bufs=6, space="PSUM"))
        po_pool = ctx.enter_context(tc.tile_pool(name="psO", bufs=2, space="PSUM"))

        # Warmup tile first (DVE memset -> PE junk MMs bridge the input DMA);
        # the tiny scalar EXP forces the ACT table load during the preamble.
        warm = const.tile([128, 640], BF16, name="warm")
        nc.vector.memset(warm[:], 0.0)
        tiny = const.tile([128, 8], BF16, name="tiny")
        nc.gpsimd.memset(tiny[:], 0.0)
        nc.scalar.activation(tiny[:, 4:8], tiny[:, 0:4], EXP, scale=SCALE)
        wps = ps_pool.tile([128, 512], F32, tag="ps", name="warmps")
        for _ in range(WARMUP_MMS):
            nc.tensor.matmul(
                wps[:], lhsT=warm[:, 0:128], rhs=warm[:, 128:640],
                start=True, stop=True,
            )

        # tribias[:, 0:128] = BIAS - BIG*[masked]; cols 128..512 = plain BIAS
        tribias = const.tile([128, 512], F32, name="tribias")
        make_upper_triangular(nc, tribias[:, 0:128], val=BIG, diag=True)
        nc.vector.tensor_scalar(
            tribias[:, 0:128], tribias[:, 0:128], 1.0, BIAS - BIG, MULT, ADD
        )
        nc.vector.memset(tribias[:, 128:512], BIAS)

        def dma_in(i, qt, kt, vt):
            for hf in range(2):
                cs = slice(1024 * hf, 1024 * hf + 1024)
                nc.sync.dma_start(kt[:, cs], kT[i][:, cs])
                nc.sync.dma_start(qt[:, cs], qT[i][:, cs])
                vs = slice(8 * 65 * hf, 8 * 65 * hf + 8 * 65)
                nc.sync.dma_start(vt[:, vs], v1[i][:, vs])

        qts, kts, vts = {}, {}, {}

        def alloc_head(i):
            qts[i] = qk_pool.tile([128, L], BF16, tag="qt", name=f"qt{i}")
            kts[i] = qk_pool.tile([128, L], BF16, tag="kt", name=f"kt{i}")
            vts[i] = v_pool.tile([128, NCH * 65], BF16, tag="vt", name=f"vt{i}")
            dma_in(i, qts[i], kts[i], vts[i])

        alloc_head(0)

        for i in range(HPC):
            qt, kt, vt = qts[i], kts[i], vts[i]

            for win in range(NWIN):
                qlo = 512 * win
                qhi = qlo + 512
                if win == 2 and i + 1 < HPC:
                    alloc_head(i + 1)  # prefetch next head's inputs

                oT = po_pool.tile([65, 512], F32, tag="oT", name=f"oT{i}_{win}")
                nlast = 4 * win + 3
                pend = []

                def emit_av(u):
                    n, q0, w, a_sb = u
                    nc.tensor.matmul(
                        oT[:, q0 - qlo : q0 - qlo + w],
                        lhsT=vt[:, n * 65 : n * 65 + 65],
                        rhs=a_sb[:, 0:w],
                        start=(n == 0),
                        stop=(n == nlast),
                    )
                    if n == nlast:
                        osb = o_pool.tile(
                            [65, 512], F32, tag="osb", name=f"osb{i}_{win}"
                        )
                        if win % 2 == 0:
                            nc.vector.tensor_copy(osb[:], oT[:])
                        else:
                            nc.scalar.activation(osb[:], oT[:], COPYF)
                        nc.sync.dma_start(out[i, win], osb[:])

                for p in range((nlast + 1) // 2):
                    group = []
                    for n in (2 * p, 2 * p + 1):
                        q0 = max(128 * n, qlo)
                        w = qhi - q0
                        ps = ps_pool.tile(
                            [128, 512], F32, tag="ps", name=f"ps{i}_{win}_{n}"
                        )
                        group.append((n, q0, w, ps))
                    # adjacent emission -> the 64-row halves run concurrently
                    for n, q0, w, ps in group:
                        half = n & 1
                        pr = slice(64 * half, 64 * half + 64)
                        nc.tensor.matmul(
                            ps[:, 0:w],
                            lhsT=kt[pr, 128 * n : 128 * n + 128],
                            rhs=qt[pr, q0:qhi],
                            start=True,
                            stop=True,
                            tile_position=(64 * half, 0),
                        )
                    for n, q0, w, ps in group:
                        a_sb = a_pool.tile(
                            [128, 512], BF16, tag="a", name=f"a{i}_{win}_{n}"
                        )
                        diag = n >= 4 * win
                        if diag or n in DVE_INT[win]:
                            if diag:
                                nc.vector.scalar_tensor_tensor(
                                    out=a_sb[:, 0:w].bitcast(I16),
                                    in0=ps[:, 0:w],
                                    scalar=C1,
                                    in1=tribias[:, 0:w],
                                    op0=MULT,
                                    op1=ADD,
                                )
                            else:
                                nc.vector.tensor_scalar(
                                    a_sb[:, 0:w].bitcast(I16),
                                    ps[:, 0:w],
                                    C1,
                                    BIAS,
                                    MULT,
                                    ADD,
                                )
                        else:
                            nc.scalar.activation(
                                a_sb[:, 0:w], ps[:, 0:w], EXP, scale=SCALE
                            )
                        pend.append((n, q0, w, a_sb))

                    while len(pend) > LAG:
                        emit_av(pend.pop(0))
                for u in pend:
                    emit_av(u)


_CACHED = None


def _build():
    global _CACHED
    if _CACHED is not None:
        return _CACHED
    nc = bacc.Bacc("TRN2", target_bir_lowering=False, debug=False)
    qT = nc.dram_tensor("qT", [HPC, 128, L], BF16, kind="ExternalInput").ap()
    kT = nc.dram_tensor("kT", [HPC, 128, L], BF16, kind="ExternalInput").ap()
    v1 = nc.dram_tensor("v1", [HPC, 128, NCH * 65], BF16, kind="ExternalInput").ap()
    out = nc.dram_tensor("out", [HPC, NWIN, 65, 512], F32, kind="ExternalOutput").ap()
    with tile.TileContext(nc) as tc:
        _body(tc, qT, kT, v1, out)
    nc.compile()
    _CACHED = nc
    return nc


def _prep_in_maps(queries, keys, values, tau, delta):
    """Shard + relayout the full inputs into 8 per-core input dicts."""
    queries = np.asarray(queries, dtype=np.float32)
    keys = np.asarray(keys, dtype=np.float32)
    values = np.asarray(values, dtype=np.float32)
    tau = np.asarray(tau, dtype=np.float32)
    delta = np.asarray(delta, dtype=np.float32)

    ktau = keys * tau[:, 0][:, None, None, None]
    D = np.exp(delta / 8.0)  # [B, L]
    vD = values * D[:, :, None, None]

    in_maps = []
    for core in range(NCORES):
        qTs = np.empty((HPC, 128, L), ml_dtypes.bfloat16)
        kTs = np.empty((HPC, 128, L), ml_dtypes.bfloat16)
        v1s = np.empty((HPC, 128, NCH * 65), ml_dtypes.bfloat16)
        for slot in range(HPC):
            g = core * HPC + slot
            b, h = divmod(g, H)
            qtv = queries[b, :, h, :].T.astype(ml_dtypes.bfloat16)
            ktv = ktau[b, :, h, :].T.astype(ml_dtypes.bfloat16)
            # duplicate across both partition halves for PE row tiling
            qTs[slot, 0:64] = qtv
            qTs[slot, 64:128] = qtv
            kTs[slot, 0:64] = ktv
            kTs[slot, 64:128] = ktv
            v = vD[b, :, h, :].reshape(NCH, 128, E).transpose(1, 0, 2)
            dd = D[b].reshape(NCH, 128).T[:, :, None]
            vv = np.concatenate([v, dd], axis=2)
            v1s[slot] = vv.reshape(128, NCH * 65).astype(ml_dtypes.bfloat16)
        in_maps.append({"qT": qTs, "kT": kTs, "v1": v1s})
    return in_maps


def _assemble(results):
    O = np.empty((B, L, H, E), np.float32)
    allo = np.stack([results[c]["out"] for c in range(NCORES)])
    # [8, HPC, NWIN, 65, 512]
    num = allo[:, :, :, 0:64, :]
    den = allo[:, :, :, 64:65, :]
    ot = num / den
    # [core, slot, win, e, col] -> [core, slot, L, E]
    ot = ot.transpose(0, 1, 2, 4, 3).reshape(NCORES, HPC, L, E)
    for core in range(NCORES):
        for slot in range(HPC):
            g = core * HPC + slot
            b, h = divmod(g, H)
            O[b, :, h, :] = ot[core, slot]
    return O


def run(inputs, trace=False, **kwargs):
    from concourse import bass_utils

    nc = _build()
    in_maps = _prep_in_maps(**inputs)
    res = bass_utils.run_bass_kernel_spmd(
        nc, in_maps, core_ids=list(range(NCORES)), trace=trace, **kwargs
    )
    return _assemble(res.results), res


def kernel(**inputs):
    return run(inputs, trace=False)[0]


# revision 11
# speedup vs baseline: 1.0747x; 1.0747x over previous
"""DSAttention Trainium2 kernel (v4: 512-col q-windows, row-tiled QK pairs).

Reference computation (per batch b, head h):
    S[q,s]  = (Q[q]·K[s]) * tau[b] + delta[b,s]
    S      += causal mask (s > q -> -inf)
    A       = softmax(S / sqrt(E), axis=s)
    O[q,:]  = sum_s A[q,s] * V[s,:]

Shapes: B=2, L=2048, H=16, E=64 -> 32 (b,h) pairs, 4 per NeuronCore x 8 cores.

Host-side algebraic folds:
  - tau folds into K:  kt = (K * tau)^T                  (bf16, dup [128, L])
  - delta folds into V: A = exp((QK*tau)/8) * D_s with D_s = exp(delta_s/8);
    O = (V*D)^T A / (D^T A), so v1 = [V*D | D] ([128, NCH, 65] chunks).

Device design:
  - q is processed in 512-col windows (4 per head).  Per window, s-chunks
    n = 0..4w+3 each produce S^T scores in their own [128,512] PSUM tile
    (1 bank): 6 tiles in the pool, so the QK->exp->AV pipeline never
    stalls on slot recycling and window boundaries refill instantly.
  - QK contraction is only E=64, so Q^T/K^T are duplicated on SBUF
    partition halves and even/odd chunks run as CONCURRENT 64-row PE
    tiles (tile_position (0,0)/(64,0)) -> ~2x QK throughput.
  - exp is assigned per-chunk to one engine: DVE does a single
    scalar_tensor_tensor Schraudolph (fused causal mask via tribias,
    output bf16 bits as saturating int16) on diagonal chunks plus some
    interior ones; ScalarE does exact exp on the rest.  Whole-chunk
    assignment keeps the ScalarE instruction count (213ns each fixed
    cost) low.
  - AV accumulates [V*D | D]^T @ A^T into a [65,512] PSUM tile; the
    denominator rides in row 64.  One PSUM->SBUF copy per window
    (alternating DVE/ScalarE), then DMA out; host divides.
  - exp ACT table preloaded via a dummy activation; ~6 junk warmup MMs
    bridge the initial DMA so the PE HAM clock gate opens early.
"""

import math
import sys

sys.path.insert(0, "/opt/trn_rl_repo")

import ml_dtypes
import numpy as np

import concourse.bass as bass
import concourse.tile as tile
from concourse import bacc, mybir
from concourse.masks import make_upper_triangular

B, L, H, E = 2, 2048, 16, 64
NCORES = 8
HPC = (B * H) // NCORES  # heads per core = 4
NCH = L // 128  # s-chunks per head = 16
NWIN = L // 512  # q-windows per head = 4
SCALE = 1.0 / 8.0  # 1/sqrt(E)
F32 = mybir.dt.float32
I16 = mybir.dt.int16
BF16 = mybir.dt.bfloat16
EXP = mybir.ActivationFunctionType.Exp
COPYF = mybir.ActivationFunctionType.Copy
MULT = mybir.AluOpType.mult
ADD = mybir.AluOpType.add

# Schraudolph constants (bf16 bit trick): bits = round(ps*C1 + BIAS)
LOG2E = 1.4426950408889634
C1 = 128.0 * LOG2E * SCALE  # 16*log2(e)
C_ADJ = 7.0  # error-centering constant
BIAS = 127.0 * 128.0 - C_ADJ
BIG = 1.0e9  # pushes masked entries to int16 saturation = bf16 -0.0

WARMUP_MMS = 10
LAG = 5  # chunks of QK/exp emitted ahead of their AV

# Interior chunks whose exp runs whole on DVE (diag chunks are always DVE).
DVE_INT = {0: frozenset(), 1: frozenset({1}), 2: frozenset({1, 5}),
           3: frozenset({1, 5, 9})}


def _body(tc, qT, kT, v1, out):
    nc = tc.nc
    from contextlib import ExitStack

    with ExitStack() as ctx:
        const = ctx.enter_context(tc.tile_pool(name="const", bufs=1))
        qk_pool = ctx.enter_context(tc.tile_pool(name="qk", bufs=2))
        v_pool = ctx.enter_context(tc.tile_pool(name="v", bufs=2))
        a_pool = ctx.enter_context(tc.tile_pool(name="a", bufs=LAG + 2))
        o_pool = ctx.enter_context(tc.tile_pool(name="o", bufs=8))
        ps_pool = ctx.enter_context(tc.tile_pool(name="psS", bufs=6, space="PSUM"))
        po_pool = ctx.enter_context(tc.tile_pool(name="psO", bufs=2, space="PSUM"))

        # Warmup tile first (GpSimd memset so it lands before the DVE iota
        # preamble finishes -> PE junk MMs start early and bridge the input
        # DMA).  The tiny scalar EXP forces the ACT table load during the
        # preamble.
        warm = const.tile([128, 640], BF16, name="warm")
        nc.gpsimd.memset(warm[:], 0.0)
        tiny = const.tile([128, 8], BF16, name="tiny")
        nc.gpsimd.memset(tiny[:], 0.0)
        nc.scalar.activation(tiny[:, 4:8], tiny[:, 0:4], EXP, scale=SCALE)
        wps = ps_pool.tile([128, 512], F32, tag="ps", name="warmps")
        for _ in range(WARMUP_MMS):
            nc.tensor.matmul(
                wps[:], lhsT=warm[:, 0:128], rhs=warm[:, 128:640],
                start=True, stop=True,
            )

        # tribias[:, 0:128] = BIAS - BIG*[masked]; cols 128..512 = plain BIAS
        tribias = const.tile([128, 512], F32, name="tribias")
        make_upper_triangular(nc, tribias[:, 0:128], val=BIG, diag=True)
        nc.vector.tensor_scalar(
            tribias[:, 0:128], tribias[:, 0:128], 1.0, BIAS - BIG, MULT, ADD
        )
        nc.vector.memset(tribias[:, 128:512], BIAS)

        def dma_in(i, qt, kt, vt):
            # Windows are processed largest-first (win 3..0), which needs all
            # of kt/vt plus qt's upper half first; qt's lower half is only
            # read from the 3rd window on.
            h0 = slice(0, 1024)
            h1 = slice(1024, 2048)
            nc.sync.dma_start(kt[:, h0], kT[i][:, h0])
            nc.sync.dma_start(kt[:, h1], kT[i][:, h1])
            nc.sync.dma_start(qt[:, h1], qT[i][:, h1])
            for hf in range(2):
                vs = slice(8 * 65 * hf, 8 * 65 * hf + 8 * 65)
                nc.sync.dma_start(vt[:, vs], v1[i][:, vs])
            nc.sync.dma_start(qt[:, h0], qT[i][:, h0])

        qts, kts, vts = {}, {}, {}

        def alloc_head(i):
            qts[i] = qk_pool.tile([128, L], BF16, tag="qt", name=f"qt{i}")
            kts[i] = qk_pool.tile([128, L], BF16, tag="kt", name=f"kt{i}")
            vts[i] = v_pool.tile([128, NCH * 65], BF16, tag="vt", name=f"vt{i}")
            dma_in(i, qts[i], kts[i], vts[i])

        alloc_head(0)

        for i in range(HPC):
            qt, kt, vt = qts[i], kts[i], vts[i]

            for win in range(NWIN - 1, -1, -1):  # largest window first
                qlo = 512 * win
                qhi = qlo + 512
                if win == 1 and i + 1 < HPC:
                    alloc_head(i + 1)  # prefetch next head's inputs

                oT = po_pool.tile([65, 512], F32, tag="oT", name=f"oT{i}_{win}")
                nlast = 4 * win + 3
                pend = []

                def emit_av(u):
                    n, q0, w, a_sb = u
                    nc.tensor.matmul(
                        oT[:, q0 - qlo : q0 - qlo + w],
                        lhsT=vt[:, n * 65 : n * 65 + 65],
                        rhs=a_sb[:, 0:w],
                        start=(n == 0),
                        stop=(n == nlast),
                    )
                    if n == nlast:
                        osb = o_pool.tile(
                            [65, 512], F32, tag="osb", name=f"osb{i}_{win}"
                        )
                        if win % 2 == 0:
                            nc.vector.tensor_copy(osb[:], oT[:])
                        else:
                            nc.scalar.activation(osb[:], oT[:], COPYF)
                        nc.sync.dma_start(out[i, win], osb[:])

                for p in range((nlast + 1) // 4):
                    group = []
                    for n in range(4 * p, 4 * p + 4):
                        q0 = max(128 * n, qlo)
                        w = qhi - q0
                        ps = ps_pool.tile(
                            [128, 512], F32, tag="ps", name=f"ps{i}_{win}_{n}"
                        )
                        group.append((n, q0, w, ps))
                    # adjacent emission -> the 64-row halves run concurrently
                    for n, q0, w, ps in group:
                        half = n & 1
                        pr = slice(64 * half, 64 * half + 64)
                        nc.tensor.matmul(
                            ps[:, 0:w],
                            lhsT=kt[pr, 128 * n : 128 * n + 128],
                            rhs=qt[pr, q0:qhi],
                            start=True,
                            stop=True,
                            tile_position=(64 * half, 0),
                        )
                    for n, q0, w, ps in group:
                        a_sb = a_pool.tile(
                            [128, 512], BF16, tag="a", name=f"a{i}_{win}_{n}"
                        )
                        diag = n >= 4 * win
                        if diag or n in DVE_INT[win]:
                            if diag:
                                nc.vector.scalar_tensor_tensor(
                                    out=a_sb[:, 0:w].bitcast(I16),
                                    in0=ps[:, 0:w],
                                    scalar=C1,
                                    in1=tribias[:, 0:w],
                                    op0=MULT,
                                    op1=ADD,
                                )
                            else:
                                nc.vector.tensor_scalar(
                                    a_sb[:, 0:w].bitcast(I16),
                                    ps[:, 0:w],
                                    C1,
                                    BIAS,
                                    MULT,
                                    ADD,
                                )
                        else:
                            nc.scalar.activation(
                                a_sb[:, 0:w], ps[:, 0:w], EXP, scale=SCALE
                            )
                        pend.append((n, q0, w, a_sb))

                    while len(pend) > LAG:
                        emit_av(pend.pop(0))
                for u in pend:
                    emit_av(u)


_CACHED = None


def _build():
    global _CACHED
    if _CACHED is not None:
        return _CACHED
    nc = bacc.Bacc("TRN2", target_bir_lowering=False, debug=False)
    qT = nc.dram_tensor("qT", [HPC, 128, L], BF16, kind="ExternalInput").ap()
    kT = nc.dram_tensor("kT", [HPC, 128, L], BF16, kind="ExternalInput").ap()
    v1 = nc.dram_tensor("v1", [HPC, 128, NCH * 65], BF16, kind="ExternalInput").ap()
    out = nc.dram_tensor("out", [HPC, NWIN, 65, 512], F32, kind="ExternalOutput").ap()
    with tile.TileContext(nc) as tc:
        _body(tc, qT, kT, v1, out)
    nc.compile()
    _CACHED = nc
    return nc


def _prep_in_maps(queries, keys, values, tau, delta):
    """Shard + relayout the full inputs into 8 per-core input dicts."""
    queries = np.asarray(queries, dtype=np.float32)
    keys = np.asarray(keys, dtype=np.float32)
    values = np.asarray(values, dtype=np.float32)
    tau = np.asarray(tau, dtype=np.float32)
    delta = np.asarray(delta, dtype=np.float32)

    ktau = keys * tau[:, 0][:, None, None, None]
    D = np.exp(delta / 8.0)  # [B, L]
    vD = values * D[:, :, None, None]

    in_maps = []
    for core in range(NCORES):
        qTs = np.empty((HPC, 128, L), ml_dtypes.bfloat16)
        kTs = np.empty((HPC, 128, L), ml_dtypes.bfloat16)
        v1s = np.empty((HPC, 128, NCH * 65), ml_dtypes.bfloat16)
        for slot in range(HPC):
            g = core * HPC + slot
            b, h = divmod(g, H)
            qtv = queries[b, :, h, :].T.astype(ml_dtypes.bfloat16)
            ktv = ktau[b, :, h, :].T.astype(ml_dtypes.bfloat16)
            # duplicate across both partition halves for PE row tiling
            qTs[slot, 0:64] = qtv
            qTs[slot, 64:128] = qtv
            kTs[slot, 0:64] = ktv
            kTs[slot, 64:128] = ktv
            v = vD[b, :, h, :].reshape(NCH, 128, E).transpose(1, 0, 2)
            dd = D[b].reshape(NCH, 128).T[:, :, None]
            vv = np.concatenate([v, dd], axis=2)
            v1s[slot] = vv.reshape(128, NCH * 65).astype(ml_dtypes.bfloat16)
        in_maps.append({"qT": qTs, "kT": kTs, "v1": v1s})
    return in_maps


def _assemble(results):
    O = np.empty((B, L, H, E), np.float32)
    allo = np.stack([results[c]["out"] for c in range(NCORES)])
    # [8, HPC, NWIN, 65, 512]
    num = allo[:, :, :, 0:64, :]
    den = allo[:, :, :, 64:65, :]
    ot = num / den
    # [core, slot, win, e, col] -> [core, slot, L, E]
    ot = ot.transpose(0, 1, 2, 4, 3).reshape(NCORES, HPC, L, E)
    for core in range(NCORES):
        for slot in range(HPC):
            g = core * HPC + slot
            b, h = divmod(g, H)
            O[b, :, h, :] = ot[core, slot]
    return O


def run(inputs, trace=False, **kwargs):
    from concourse import bass_utils

    nc = _build()
    in_maps = _prep_in_maps(**inputs)
    res = bass_utils.run_bass_kernel_spmd(
        nc, in_maps, core_ids=list(range(NCORES)), trace=trace, **kwargs
    )
    return _assemble(res.results), res


def kernel(**inputs):
    return run(inputs, trace=False)[0]


# revision 19
# speedup vs baseline: 1.0944x; 1.0184x over previous
"""DSAttention Trainium2 kernel (512-col q-windows, row-tiled QK pairs).

Reference computation (per batch b, head h):
    S[q,s]  = (Q[q]·K[s]) * tau[b] + delta[b,s]
    S      += causal mask (s > q -> -inf)
    A       = softmax(S / sqrt(E), axis=s)
    O[q,:]  = sum_s A[q,s] * V[s,:]

Shapes: B=2, L=2048, H=16, E=64 -> 32 (b,h) pairs, 4 per NeuronCore x 8 cores.

Host-side algebraic folds:
  - tau folds into K:  kt = (K * tau)^T                  (bf16, dup [128, L])
  - delta folds into V: A = exp((QK*tau)/8) * D_s with D_s = exp(delta_s/8);
    O = (V*D)^T A / (D^T A), so v1 = [V*D | D] ([128, NCH, 65] chunks).

Device design:
  - q is processed in 512-col windows (4 per head).  Per window, s-chunks
    n = 0..4w+3 each produce S^T scores in their own [128,512] PSUM tile
    (1 bank): 6 tiles in the pool, so the QK->exp->AV pipeline never
    stalls on slot recycling and window boundaries refill instantly.
  - QK contraction is only E=64, so Q^T/K^T are duplicated on SBUF
    partition halves and even/odd chunks run as CONCURRENT 64-row PE
    tiles (tile_position (0,0)/(64,0)) -> ~2x QK throughput.
  - exp is assigned per-chunk to one engine: DVE does a single
    scalar_tensor_tensor Schraudolph (fused causal mask via tribias,
    output bf16 bits as saturating int16) on diagonal chunks plus some
    interior ones; ScalarE does exact exp on the rest.  Whole-chunk
    assignment keeps the ScalarE instruction count (213ns each fixed
    cost) low.
  - AV accumulates [V*D | D]^T @ A^T into a [65,512] PSUM tile; the
    denominator rides in row 64.  One PSUM->SBUF copy per window
    (alternating DVE/ScalarE), then DMA out; host divides.
  - exp ACT table preloaded via a dummy activation; 8 junk warmup MMs
    bridge the initial DMA so the PE HAM clock gate opens before real
    work arrives.  Windows are processed largest-first so dense
    full-array work comes early (HAM) and the kernel tail is the
    smallest window.
"""

import math
import sys

sys.path.insert(0, "/opt/trn_rl_repo")

import ml_dtypes
import numpy as np

import concourse.bass as bass
import concourse.tile as tile
from concourse import bacc, mybir
from concourse.masks import make_upper_triangular

B, L, H, E = 2, 2048, 16, 64
NCORES = 8
HPC = (B * H) // NCORES  # heads per core = 4
NCH = L // 128  # s-chunks per head = 16
NWIN = L // 512  # q-windows per head = 4
SCALE = 1.0 / 8.0  # 1/sqrt(E)
F32 = mybir.dt.float32
I16 = mybir.dt.int16
BF16 = mybir.dt.bfloat16
EXP = mybir.ActivationFunctionType.Exp
COPYF = mybir.ActivationFunctionType.Copy
MULT = mybir.AluOpType.mult
ADD = mybir.AluOpType.add

# Schraudolph constants (bf16 bit trick): bits = round(ps*C1 + BIAS)
LOG2E = 1.4426950408889634
C1 = 128.0 * LOG2E * SCALE  # 16*log2(e)
C_ADJ = 7.0  # error-centering constant
BIAS = 127.0 * 128.0 - C_ADJ
BIG = 1.0e9  # pushes masked entries to int16 saturation = bf16 -0.0

WARMUP_MMS = 8
LAG = 5  # chunks of QK/exp emitted ahead of their AV

# Interior chunks whose exp runs whole on DVE (diag chunks are always DVE).
DVE_INT = {0: frozenset(), 1: frozenset({1}), 2: frozenset({1, 5}),
           3: frozenset({1, 5, 9})}


def _body(tc, qT, kT, v1, out):
    nc = tc.nc
    from contextlib import ExitStack

    with ExitStack() as ctx:
        const = ctx.enter_context(tc.tile_pool(name="const", bufs=1))
        qk_pool = ctx.enter_context(tc.tile_pool(name="qk", bufs=2))
        v_pool = ctx.enter_context(tc.tile_pool(name="v", bufs=2))
        a_pool = ctx.enter_context(tc.tile_pool(name="a", bufs=LAG + 2))
        o_pool = ctx.enter_context(tc.tile_pool(name="o", bufs=8))
        ps_pool = ctx.enter_context(tc.tile_pool(name="psS", bufs=6, space="PSUM"))
        po_pool = ctx.enter_context(tc.tile_pool(name="psO", bufs=2, space="PSUM"))

        # Warmup tile first (GpSimd memset) -> PE junk MMs bridge the input
        # DMA wait and open the HAM clock gate before real work arrives.
        # The tiny scalar EXP forces the ACT table load during the preamble.
        warm = const.tile([128, 640], BF16, name="warm")
        nc.vector.memset(warm[:], 0.0)
        tiny = const.tile([128, 8], BF16, name="tiny")
        nc.gpsimd.memset(tiny[:], 0.0)
        nc.scalar.activation(tiny[:, 4:8], tiny[:, 0:4], EXP, scale=SCALE)
        wps = ps_pool.tile([128, 512], F32, tag="ps", name="warmps")
        for _ in range(WARMUP_MMS):
            nc.tensor.matmul(
                wps[:], lhsT=warm[:, 0:128], rhs=warm[:, 128:640],
                start=True, stop=True,
            )

        # tribias[:, 0:128] = BIAS - BIG*[masked]; cols 128..512 = plain BIAS
        tribias = const.tile([128, 512], F32, name="tribias")
        make_upper_triangular(nc, tribias[:, 0:128], val=BIG, diag=True)
        nc.vector.tensor_scalar(
            tribias[:, 0:128], tribias[:, 0:128], 1.0, BIAS - BIG, MULT, ADD
        )
        nc.vector.memset(tribias[:, 128:512], BIAS)

        def dma_in(i, qt, kt, vt):
            # Windows are processed largest-first (win 3..0), which needs all
            # of kt/vt plus qt's upper half first; qt's lower half is only
            # read from the 3rd window on.
            h0 = slice(0, 1024)
            h1 = slice(1024, 2048)
            nc.sync.dma_start(qt[:, h1], qT[i][:, h1])
            nc.sync.dma_start(kt[:, h0], kT[i][:, h0])
            nc.sync.dma_start(kt[:, h1], kT[i][:, h1])
            for hf in range(2):
                vs = slice(8 * 65 * hf, 8 * 65 * hf + 8 * 65)
                nc.sync.dma_start(vt[:, vs], v1[i][:, vs])
            nc.sync.dma_start(qt[:, h0], qT[i][:, h0])

        qts, kts, vts = {}, {}, {}

        def alloc_head(i):
            qts[i] = qk_pool.tile([128, L], BF16, tag="qt", name=f"qt{i}")
            kts[i] = qk_pool.tile([128, L], BF16, tag="kt", name=f"kt{i}")
            vts[i] = v_pool.tile([128, NCH * 65], BF16, tag="vt", name=f"vt{i}")
            dma_in(i, qts[i], kts[i], vts[i])

        alloc_head(0)

        for i in range(HPC):
            qt, kt, vt = qts[i], kts[i], vts[i]

            for win in range(NWIN - 1, -1, -1):  # largest window first
                qlo = 512 * win
                qhi = qlo + 512
                if win == 2 and i + 1 < HPC:
                    alloc_head(i + 1)  # prefetch next head's inputs

                oT = po_pool.tile([65, 512], F32, tag="oT", name=f"oT{i}_{win}")
                nlast = 4 * win + 3
                pend = []

                def emit_av(u):
                    n, q0, w, a_sb = u
                    nc.tensor.matmul(
                        oT[:, q0 - qlo : q0 - qlo + w],
                        lhsT=vt[:, n * 65 : n * 65 + 65],
                        rhs=a_sb[:, 0:w],
                        start=(n == 0),
                        stop=(n == nlast),
                    )
                    if n == nlast:
                        osb = o_pool.tile(
                            [65, 512], F32, tag="osb", name=f"osb{i}_{win}"
                        )
                        if win % 2 == 0:
                            nc.vector.tensor_copy(osb[:], oT[:])
                        else:
                            nc.scalar.activation(osb[:], oT[:], COPYF)
                        nc.sync.dma_start(out[i, win], osb[:])

                for p in range((nlast + 1) // 4):
                    group = []
                    for n in range(4 * p, 4 * p + 4):
                        q0 = max(128 * n, qlo)
                        w = qhi - q0
                        ps = ps_pool.tile(
                            [128, 512], F32, tag="ps", name=f"ps{i}_{win}_{n}"
                        )
                        group.append((n, q0, w, ps))
                    # adjacent emission -> the 64-row halves run concurrently
                    for n, q0, w, ps in group:
                        half = n & 1
                        pr = slice(64 * half, 64 * half + 64)
                        nc.tensor.matmul(
                            ps[:, 0:w],
                            lhsT=kt[pr, 128 * n : 128 * n + 128],
                            rhs=qt[pr, q0:qhi],
                            start=True,
                            stop=True,
                            tile_position=(64 * half, 0),
                        )
                    for n, q0, w, ps in group:
                        a_sb = a_pool.tile(
                            [128, 512], BF16, tag="a", name=f"a{i}_{win}_{n}"
                        )
                        diag = n >= 4 * win
                        if diag or n in DVE_INT[win]:
                            if diag:
                                nc.vector.scalar_tensor_tensor(
                                    out=a_sb[:, 0:w].bitcast(I16),
                                    in0=ps[:, 0:w],
                                    scalar=C1,
                                    in1=tribias[:, 0:w],
                                    op0=MULT,
                                    op1=ADD,
                                )
                            else:
                                nc.vector.tensor_scalar(
                                    a_sb[:, 0:w].bitcast(I16),
                                    ps[:, 0:w],
                                    C1,
                                    BIAS,
                                    MULT,
                                    ADD,
                                )
                        else:
                            nc.scalar.activation(
                                a_sb[:, 0:w], ps[:, 0:w], EXP, scale=SCALE
                            )
                        pend.append((n, q0, w, a_sb))

                    while len(pend) > LAG:
                        emit_av(pend.pop(0))
                for u in pend:
                    emit_av(u)


_CACHED = None


def _build():
    global _CACHED
    if _CACHED is not None:
        return _CACHED
    nc = bacc.Bacc("TRN2", target_bir_lowering=False, debug=False)
    qT = nc.dram_tensor("qT", [HPC, 128, L], BF16, kind="ExternalInput").ap()
    kT = nc.dram_tensor("kT", [HPC, 128, L], BF16, kind="ExternalInput").ap()
    v1 = nc.dram_tensor("v1", [HPC, 128, NCH * 65], BF16, kind="ExternalInput").ap()
    out = nc.dram_tensor("out", [HPC, NWIN, 65, 512], F32, kind="ExternalOutput").ap()
    with tile.TileContext(nc) as tc:
        _body(tc, qT, kT, v1, out)
    nc.compile()
    _CACHED = nc
    return nc


def _prep_in_maps(queries, keys, values, tau, delta):
    """Shard + relayout the full inputs into 8 per-core input dicts."""
    queries = np.asarray(queries, dtype=np.float32)
    keys = np.asarray(keys, dtype=np.float32)
    values = np.asarray(values, dtype=np.float32)
    tau = np.asarray(tau, dtype=np.float32)
    delta = np.asarray(delta, dtype=np.float32)

    ktau = keys * tau[:, 0][:, None, None, None]
    D = np.exp(delta / 8.0)  # [B, L]
    vD = values * D[:, :, None, None]

    in_maps = []
    for core in range(NCORES):
        qTs = np.empty((HPC, 128, L), ml_dtypes.bfloat16)
        kTs = np.empty((HPC, 128, L), ml_dtypes.bfloat16)
        v1s = np.empty((HPC, 128, NCH * 65), ml_dtypes.bfloat16)
        for slot in range(HPC):
            g = core * HPC + slot
            b, h = divmod(g, H)
            qtv = queries[b, :, h, :].T.astype(ml_dtypes.bfloat16)
            ktv = ktau[b, :, h, :].T.astype(ml_dtypes.bfloat16)
            # duplicate across both partition halves for PE row tiling
            qTs[slot, 0:64] = qtv
            qTs[slot, 64:128] = qtv
            kTs[slot, 0:64] = ktv
            kTs[slot, 64:128] = ktv
            v = vD[b, :, h, :].reshape(NCH, 128, E).transpose(1, 0, 2)
            dd = D[b].reshape(NCH, 128).T[:, :, None]
            vv = np.concatenate([v, dd], axis=2)
            v1s[slot] = vv.reshape(128, NCH * 65).astype(ml_dtypes.bfloat16)
        in_maps.append({"qT": qTs, "kT": kTs, "v1": v1s})
    return in_maps


def _assemble(results):
    O = np.empty((B, L, H, E), np.float32)
    allo = np.stack([results[c]["out"] for c in range(NCORES)])
    # [8, HPC, NWIN, 65, 512]
    num = allo[:, :, :, 0:64, :]
    den = allo[:, :, :, 64:65, :]
    ot = num / den
    # [core, slot, win, e, col] -> [core, slot, L, E]
    ot = ot.transpose(0, 1, 2, 4, 3).reshape(NCORES, HPC, L, E)
    for core in range(NCORES):
        for slot in range(HPC):
            g = core * HPC + slot
            b, h = divmod(g, H)
            O[b, :, h, :] = ot[core, slot]
    return O


def run(inputs, trace=False, **kwargs):
    from concourse import bass_utils

    nc = _build()
    in_maps = _prep_in_maps(**inputs)
    res = bass_utils.run_bass_kernel_spmd(
        nc, in_maps, core_ids=list(range(NCORES)), trace=trace, **kwargs
    )
    return _assemble(res.results), res


def kernel(**inputs):
    return run(inputs, trace=False)[0]


# revision 28
# speedup vs baseline: 1.3741x; 1.2555x over previous
"""DSAttention Trainium2 kernel (512-col q-windows, row-tiled QK pairs).

Reference computation (per batch b, head h):
    S[q,s]  = (Q[q]·K[s]) * tau[b] + delta[b,s]
    S      += causal mask (s > q -> -inf)
    A       = softmax(S / sqrt(E), axis=s)
    O[q,:]  = sum_s A[q,s] * V[s,:]

Shapes: B=2, L=2048, H=16, E=64 -> 32 (b,h) pairs, 4 per NeuronCore x 8 cores.

Host-side algebraic folds:
  - tau folds into K:  kt = (K * tau)^T                  (bf16, dup [128, L])
  - delta folds into V: A = exp((QK*tau)/8) * D_s with D_s = exp(delta_s/8);
    O = (V*D)^T A / (D^T A), so v1 = [V*D | D] ([128, NCH, 65] chunks).

Device design:
  - q is processed in 512-col windows (4 per head).  Per window, s-chunks
    n = 0..4w+3 each produce S^T scores in their own [128,512] PSUM tile
    (1 bank): 6 tiles in the pool, so the QK->exp->AV pipeline never
    stalls on slot recycling and window boundaries refill instantly.
  - QK contraction is only E=64, so Q^T/K^T are duplicated on SBUF
    partition halves and even/odd chunks run as CONCURRENT 64-row PE
    tiles (tile_position (0,0)/(64,0)) -> ~2x QK throughput.
  - exp is assigned per-chunk to one engine: DVE does a single
    scalar_tensor_tensor Schraudolph (fused causal mask via tribias,
    output bf16 bits as saturating int16) on diagonal chunks plus some
    interior ones; ScalarE does exact exp on the rest.  Whole-chunk
    assignment keeps the ScalarE instruction count (213ns each fixed
    cost) low.
  - AV accumulates [V*D | D]^T @ A^T into a [65,512] PSUM tile; the
    denominator rides in row 64.  One PSUM->SBUF copy per window
    (alternating DVE/ScalarE), then DMA out; host divides.
  - exp ACT table preloaded via a dummy activation; 8 junk warmup MMs
    bridge the initial DMA so the PE HAM clock gate opens before real
    work arrives.  Windows are processed largest-first so dense
    full-array work comes early (HAM) and the kernel tail is the
    smallest window.
"""

import math
import sys

sys.path.insert(0, "/opt/trn_rl_repo")

import ml_dtypes
import numpy as np

import concourse.bass as bass
import concourse.tile as tile
from concourse import bacc, mybir
from concourse.masks import make_upper_triangular

B, L, H, E = 2, 2048, 16, 64
NCORES = 8
HPC = (B * H) // NCORES  # heads per core = 4
NCH = L // 128  # s-chunks per head = 16
NWIN = L // 512  # q-windows per head = 4
SCALE = 1.0 / 8.0  # 1/sqrt(E)
F32 = mybir.dt.float32
I16 = mybir.dt.int16
BF16 = mybir.dt.bfloat16
EXP = mybir.ActivationFunctionType.Exp
COPYF = mybir.ActivationFunctionType.Copy
MULT = mybir.AluOpType.mult
ADD = mybir.AluOpType.add

# Schraudolph constants (bf16 bit trick): bits = round(ps*C1 + BIAS)
LOG2E = 1.4426950408889634
C1 = 128.0 * LOG2E * SCALE  # 16*log2(e)
C_ADJ = 7.0  # error-centering constant
BIAS = 127.0 * 128.0 - C_ADJ
BIG = 1.0e9  # pushes masked entries to int16 saturation = bf16 -0.0

WARMUP_MMS = 8
LAG = 5  # chunks of QK/exp emitted ahead of their AV

# exp engine split: per 4-chunk group, 2 chunks on DVE and 2 on ScalarE so
# neither engine drags the group cycle (lockstep balance beats aggregate
# rate matching).  Diag groups: the two small chunks (j=2,3) go whole to
# DVE, the two large ones (j=0,1) leave their mask stub on DVE and the
# rest on ScalarE.


def _body(tc, qT, kT, v1, out):
    nc = tc.nc
    from contextlib import ExitStack

    with ExitStack() as ctx:
        const = ctx.enter_context(tc.tile_pool(name="const", bufs=1))
        qk_pool = ctx.enter_context(tc.tile_pool(name="qk", bufs=2))
        v_pool = ctx.enter_context(tc.tile_pool(name="v", bufs=2))
        a_pool = ctx.enter_context(tc.tile_pool(name="a", bufs=LAG + 2))
        o_pool = ctx.enter_context(tc.tile_pool(name="o", bufs=8))
        ps_pool = ctx.enter_context(tc.tile_pool(name="psS", bufs=6, space="PSUM"))
        po_pool = ctx.enter_context(tc.tile_pool(name="psO", bufs=2, space="PSUM"))

        # Warmup tile first (GpSimd memset) -> PE junk MMs bridge the input
        # DMA wait and open the HAM clock gate before real work arrives.
        # The tiny scalar EXP forces the ACT table load during the preamble.
        warm = const.tile([128, 640], BF16, name="warm")
        nc.vector.memset(warm[:], 0.0)
        tiny = const.tile([128, 8], BF16, name="tiny")
        nc.gpsimd.memset(tiny[:], 0.0)
        nc.scalar.activation(tiny[:, 4:8], tiny[:, 0:4], EXP, scale=SCALE)
        wps = ps_pool.tile([128, 512], F32, tag="ps", name="warmps")
        for _ in range(WARMUP_MMS):
            nc.tensor.matmul(
                wps[:], lhsT=warm[:, 0:128], rhs=warm[:, 128:640],
                start=True, stop=True,
            )

        # tribias[:, 0:128] = BIAS - BIG*[masked]; cols 128..512 = plain BIAS
        tribias = const.tile([128, 512], F32, name="tribias")
        make_upper_triangular(nc, tribias[:, 0:128], val=BIG, diag=True)
        nc.vector.tensor_scalar(
            tribias[:, 0:128], tribias[:, 0:128], 1.0, BIAS - BIG, MULT, ADD
        )
        nc.vector.memset(tribias[:, 128:512], BIAS)

        def dma_in(i, qt, kt, vt):
            # Windows are processed largest-first (win 3..0), which needs all
            # of kt/vt plus qt's upper half first; qt's lower half is only
            # read from the 3rd window on.
            h0 = slice(0, 1024)
            h1 = slice(1024, 2048)
            nc.sync.dma_start(qt[:, h1], qT[i][:, h1])
            nc.sync.dma_start(kt[:, h0], kT[i][:, h0])
            nc.sync.dma_start(kt[:, h1], kT[i][:, h1])
            for hf in range(2):
                vs = slice(8 * 65 * hf, 8 * 65 * hf + 8 * 65)
                nc.sync.dma_start(vt[:, vs], v1[i][:, vs])
            nc.sync.dma_start(qt[:, h0], qT[i][:, h0])

        qts, kts, vts = {}, {}, {}

        def alloc_head(i):
            qts[i] = qk_pool.tile([128, L], BF16, tag="qt", name=f"qt{i}")
            kts[i] = qk_pool.tile([128, L], BF16, tag="kt", name=f"kt{i}")
            vts[i] = v_pool.tile([128, NCH * 65], BF16, tag="vt", name=f"vt{i}")
            dma_in(i, qts[i], kts[i], vts[i])

        alloc_head(0)

        # AV jobs trail the QK/exp stream by LAG chunks GLOBALLY (across
        # window and head boundaries): the in-order PE queue would otherwise
        # head-block on a window's final exp-gated AVs while the next
        # window's ready QKs sit behind them.
        pend = []

        def emit_av(u):
            n, q0, w, a_sb, oT, qlo2, nlast2, vt2, ii, win2 = u
            nc.tensor.matmul(
                oT[:, q0 - qlo2 : q0 - qlo2 + w],
                lhsT=vt2[:, n * 65 : n * 65 + 65],
                rhs=a_sb[:, 0:w],
                start=(n == 0),
                stop=(n == nlast2),
            )
            if n == nlast2:
                osb = o_pool.tile(
                    [65, 512], F32, tag="osb", name=f"osb{ii}_{win2}"
                )
                if win2 == 0:
                    nc.vector.tensor_copy(osb[:], oT[:])
                else:
                    nc.scalar.activation(osb[:], oT[:], COPYF)
                nc.sync.dma_start(out[ii, win2], osb[:])

        for i in range(HPC):
            qt, kt, vt = qts[i], kts[i], vts[i]

            for win in range(NWIN - 1, -1, -1):  # largest window first
                qlo = 512 * win
                qhi = qlo + 512
                if win == 2 and i + 1 < HPC:
                    alloc_head(i + 1)  # prefetch next head's inputs

                oT = po_pool.tile([65, 512], F32, tag="oT", name=f"oT{i}_{win}")
                nlast = 4 * win + 3

                for p in range((nlast + 1) // 4):
                    group = []
                    for n in range(4 * p, 4 * p + 4):
                        q0 = max(128 * n, qlo)
                        w = qhi - q0
                        ps = ps_pool.tile(
                            [128, 512], F32, tag="ps", name=f"ps{i}_{win}_{n}"
                        )
                        group.append((n, q0, w, ps))
                    # adjacent emission -> the 64-row halves run concurrently
                    for n, q0, w, ps in group:
                        half = n & 1
                        pr = slice(64 * half, 64 * half + 64)
                        nc.tensor.matmul(
                            ps[:, 0:w],
                            lhsT=kt[pr, 128 * n : 128 * n + 128],
                            rhs=qt[pr, q0:qhi],
                            start=True,
                            stop=True,
                            tile_position=(64 * half, 0),
                        )
                    for n, q0, w, ps in group:
                        a_sb = a_pool.tile(
                            [128, 512], BF16, tag="a", name=f"a{i}_{win}_{n}"
                        )
                        diag = n >= 4 * win
                        if diag:
                            dve_full = (n - 4 * win) >= 2
                            x = w if dve_full else 128
                            nc.vector.scalar_tensor_tensor(
                                out=a_sb[:, 0:x].bitcast(I16),
                                in0=ps[:, 0:x],
                                scalar=C1,
                                in1=tribias[:, 0:x],
                                op0=MULT,
                                op1=ADD,
                            )
                            if x < w:
                                nc.scalar.activation(
                                    a_sb[:, x:w], ps[:, x:w], EXP, scale=SCALE
                                )
                        elif n % 2 == 0:
                            nc.vector.tensor_scalar(
                                a_sb[:, 0:w].bitcast(I16),
                                ps[:, 0:w],
                                C1,
                                BIAS,
                                MULT,
                                ADD,
                            )
                        else:
                            nc.scalar.activation(
                                a_sb[:, 0:w], ps[:, 0:w], EXP, scale=SCALE
                            )
                        pend.append((n, q0, w, a_sb, oT, qlo, nlast, vt, i, win))

                    while len(pend) > LAG:
                        emit_av(pend.pop(0))
        for u in pend:
            emit_av(u)


_CACHED = None


def _build():
    global _CACHED
    if _CACHED is not None:
        return _CACHED
    nc = bacc.Bacc("TRN2", target_bir_lowering=False, debug=False)
    qT = nc.dram_tensor("qT", [HPC, 128, L], BF16, kind="ExternalInput").ap()
    kT = nc.dram_tensor("kT", [HPC, 128, L], BF16, kind="ExternalInput").ap()
    v1 = nc.dram_tensor("v1", [HPC, 128, NCH * 65], BF16, kind="ExternalInput").ap()
    out = nc.dram_tensor("out", [HPC, NWIN, 65, 512], F32, kind="ExternalOutput").ap()
    with tile.TileContext(nc) as tc:
        _body(tc, qT, kT, v1, out)
    nc.compile()
    _CACHED = nc
    return nc


def _prep_in_maps(queries, keys, values, tau, delta):
    """Shard + relayout the full inputs into 8 per-core input dicts."""
    queries = np.asarray(queries, dtype=np.float32)
    keys = np.asarray(keys, dtype=np.float32)
    values = np.asarray(values, dtype=np.float32)
    tau = np.asarray(tau, dtype=np.float32)
    delta = np.asarray(delta, dtype=np.float32)

    ktau = keys * tau[:, 0][:, None, None, None]
    D = np.exp(delta / 8.0)  # [B, L]
    vD = values * D[:, :, None, None]

    in_maps = []
    for core in range(NCORES):
        qTs = np.empty((HPC, 128, L), ml_dtypes.bfloat16)
        kTs = np.empty((HPC, 128, L), ml_dtypes.bfloat16)
        v1s = np.empty((HPC, 128, NCH * 65), ml_dtypes.bfloat16)
        for slot in range(HPC):
            g = core * HPC + slot
            b, h = divmod(g, H)
            qtv = queries[b, :, h, :].T.astype(ml_dtypes.bfloat16)
            ktv = ktau[b, :, h, :].T.astype(ml_dtypes.bfloat16)
            # duplicate across both partition halves for PE row tiling
            qTs[slot, 0:64] = qtv
            qTs[slot, 64:128] = qtv
            kTs[slot, 0:64] = ktv
            kTs[slot, 64:128] = ktv
            v = vD[b, :, h, :].reshape(NCH, 128, E).transpose(1, 0, 2)
            dd = D[b].reshape(NCH, 128).T[:, :, None]
            vv = np.concatenate([v, dd], axis=2)
            v1s[slot] = vv.reshape(128, NCH * 65).astype(ml_dtypes.bfloat16)
        in_maps.append({"qT": qTs, "kT": kTs, "v1": v1s})
    return in_maps


def _assemble(results):
    O = np.empty((B, L, H, E), np.float32)
    allo = np.stack([results[c]["out"] for c in range(NCORES)])
    # [8, HPC, NWIN, 65, 512]
    num = allo[:, :, :, 0:64, :]
    den = allo[:, :, :, 64:65, :]
    ot = num / den
    # [core, slot, win, e, col] -> [core, slot, L, E]
    ot = ot.transpose(0, 1, 2, 4, 3).reshape(NCORES, HPC, L, E)
    for core in range(NCORES):
        for slot in range(HPC):
            g = core * HPC + slot
            b, h = divmod(g, H)
            O[b, :, h, :] = ot[core, slot]
    return O


def run(inputs, trace=False, **kwargs):
    from concourse import bass_utils

    nc = _build()
    in_maps = _prep_in_maps(**inputs)
    res = bass_utils.run_bass_kernel_spmd(
        nc, in_maps, core_ids=list(range(NCORES)), trace=trace, **kwargs
    )
    return _assemble(res.results), res


def kernel(**inputs):
    return run(inputs, trace=False)[0]


# revision 38
# speedup vs baseline: 1.4347x; 1.0441x over previous
"""DSAttention Trainium2 kernel (512-col q-windows, row-tiled QK pairs).

Reference computation (per batch b, head h):
    S[q,s]  = (Q[q]·K[s]) * tau[b] + delta[b,s]
    S      += causal mask (s > q -> -inf)
    A       = softmax(S / sqrt(E), axis=s)
    O[q,:]  = sum_s A[q,s] * V[s,:]

Shapes: B=2, L=2048, H=16, E=64 -> 32 (b,h) pairs, 4 per NeuronCore x 8 cores.

Host-side algebraic folds:
  - tau folds into K:  kt = (K * tau)^T                  (bf16, dup [128, L])
  - delta folds into V: A = exp((QK*tau)/8) * D_s with D_s = exp(delta_s/8);
    O = (V*D)^T A / (D^T A), so v1 = [V*D | D] ([128, NCH, 65] chunks).

Device design:
  - q is processed in 512-col windows (4 per head).  Per window, s-chunks
    n = 0..4w+3 each produce S^T scores in their own [128,512] PSUM tile
    (1 bank): 6 tiles in the pool, so the QK->exp->AV pipeline never
    stalls on slot recycling and window boundaries refill instantly.
  - QK contraction is only E=64, so Q^T/K^T are duplicated on SBUF
    partition halves and even/odd chunks run as CONCURRENT 64-row PE
    tiles (tile_position (0,0)/(64,0)) -> ~2x QK throughput.
  - exp is assigned per-chunk to one engine: DVE does a single
    scalar_tensor_tensor Schraudolph (fused causal mask via tribias,
    output bf16 bits as saturating int16) on diagonal chunks plus some
    interior ones; ScalarE does exact exp on the rest.  Whole-chunk
    assignment keeps the ScalarE instruction count (213ns each fixed
    cost) low.
  - AV accumulates [V*D | D]^T @ A^T into a [65,512] PSUM tile; the
    denominator rides in row 64.  One PSUM->SBUF copy per window
    (alternating DVE/ScalarE), then DMA out; host divides.
  - exp ACT table preloaded via a dummy activation; 8 junk warmup MMs
    bridge the initial DMA so the PE HAM clock gate opens before real
    work arrives.  Windows are processed largest-first so dense
    full-array work comes early (HAM) and the kernel tail is the
    smallest window.
"""

import math
import sys

sys.path.insert(0, "/opt/trn_rl_repo")

import ml_dtypes
import numpy as np

import concourse.bass as bass
import concourse.tile as tile
from concourse import bacc, mybir
from concourse.masks import make_upper_triangular

B, L, H, E = 2, 2048, 16, 64
NCORES = 8
HPC = (B * H) // NCORES  # heads per core = 4
NCH = L // 128  # s-chunks per head = 16
NWIN = L // 512  # q-windows per head = 4
SCALE = 1.0 / 8.0  # 1/sqrt(E)
F32 = mybir.dt.float32
I16 = mybir.dt.int16
BF16 = mybir.dt.bfloat16
EXP = mybir.ActivationFunctionType.Exp
COPYF = mybir.ActivationFunctionType.Copy
MULT = mybir.AluOpType.mult
ADD = mybir.AluOpType.add

# Schraudolph constants (bf16 bit trick): bits = round(ps*C1 + BIAS)
LOG2E = 1.4426950408889634
C1 = 128.0 * LOG2E * SCALE  # 16*log2(e)
C_ADJ = 7.0  # error-centering constant
BIAS = 127.0 * 128.0 - C_ADJ
BIG = 1.0e9  # pushes masked entries to int16 saturation = bf16 -0.0

WARMUP_MMS = 8
LAG = 5  # chunks of QK/exp emitted ahead of their AV

# exp engine split: per 4-chunk group, 2 chunks on DVE and 2 on ScalarE so
# neither engine drags the group cycle (lockstep balance beats aggregate
# rate matching).  Diag groups: the two small chunks (j=2,3) go whole to
# DVE, the two large ones (j=0,1) leave their mask stub on DVE and the
# rest on ScalarE.


def _body(tc, qT, kT, v1, out):
    nc = tc.nc
    from contextlib import ExitStack

    with ExitStack() as ctx:
        const = ctx.enter_context(tc.tile_pool(name="const", bufs=1))
        qk_pool = ctx.enter_context(tc.tile_pool(name="qk", bufs=2))
        v_pool = ctx.enter_context(tc.tile_pool(name="v", bufs=2))
        a_pool = ctx.enter_context(tc.tile_pool(name="a", bufs=LAG + 2))
        o_pool = ctx.enter_context(tc.tile_pool(name="o", bufs=8))
        ps_pool = ctx.enter_context(tc.tile_pool(name="psS", bufs=6, space="PSUM"))
        po_pool = ctx.enter_context(tc.tile_pool(name="psO", bufs=2, space="PSUM"))

        # Warmup tile first (GpSimd memset) -> PE junk MMs bridge the input
        # DMA wait and open the HAM clock gate before real work arrives.
        # The tiny scalar EXP forces the ACT table load during the preamble.
        warm = const.tile([128, 640], BF16, name="warm")
        nc.vector.memset(warm[:], 0.0)
        tiny = const.tile([128, 8], BF16, name="tiny")
        nc.gpsimd.memset(tiny[:], 0.0)
        nc.scalar.activation(tiny[:, 4:8], tiny[:, 0:4], EXP, scale=SCALE)
        wps = ps_pool.tile([128, 512], F32, tag="ps", name="warmps")
        for _ in range(WARMUP_MMS):
            nc.tensor.matmul(
                wps[:], lhsT=warm[:, 0:128], rhs=warm[:, 128:640],
                start=True, stop=True,
            )

        # tribias[:, 0:128] = BIAS - BIG*[masked]; cols 128..512 = plain BIAS
        tribias = const.tile([128, 512], F32, name="tribias")
        make_upper_triangular(nc, tribias[:, 0:128], val=BIG, diag=True)
        nc.vector.tensor_scalar(
            tribias[:, 0:128], tribias[:, 0:128], 1.0, BIAS - BIG, MULT, ADD
        )
        nc.vector.memset(tribias[:, 128:512], BIAS)

        def dma_in(i, qt, kt, vt):
            # Windows are processed largest-first (win 3..0), which needs all
            # of kt/vt plus qt's upper half first; qt's lower half is only
            # read from the 3rd window on.
            h0 = slice(0, 1024)
            h1 = slice(1024, 2048)
            nc.sync.dma_start(qt[:, h1], qT[i][:, h1])
            nc.sync.dma_start(kt[:, h0], kT[i][:, h0])
            nc.sync.dma_start(kt[:, h1], kT[i][:, h1])
            for hf in range(2):
                vs = slice(8 * 65 * hf, 8 * 65 * hf + 8 * 65)
                nc.sync.dma_start(vt[:, vs], v1[i][:, vs])
            nc.sync.dma_start(qt[:, h0], qT[i][:, h0])

        qts, kts, vts = {}, {}, {}

        def alloc_head(i):
            qts[i] = qk_pool.tile([128, L], BF16, tag="qt", name=f"qt{i}")
            kts[i] = qk_pool.tile([128, L], BF16, tag="kt", name=f"kt{i}")
            vts[i] = v_pool.tile([128, NCH * 65], BF16, tag="vt", name=f"vt{i}")
            dma_in(i, qts[i], kts[i], vts[i])

        alloc_head(0)

        # AV jobs trail the QK/exp stream by LAG chunks GLOBALLY (across
        # window and head boundaries): the in-order PE queue would otherwise
        # head-block on a window's final exp-gated AVs while the next
        # window's ready QKs sit behind them.
        pend = []

        def emit_av(u):
            n, q0, w, a_sb, oT, qlo2, nlast2, vt2, ii, win2 = u
            nc.tensor.matmul(
                oT[:, q0 - qlo2 : q0 - qlo2 + w],
                lhsT=vt2[:, n * 65 : n * 65 + 65],
                rhs=a_sb[:, 0:w],
                start=(n == 0),
                stop=(n == nlast2),
            )
            if n == nlast2:
                osb = o_pool.tile(
                    [65, 512], F32, tag="osb", name=f"osb{ii}_{win2}"
                )
                if win2 == 0:
                    nc.vector.tensor_copy(osb[:], oT[:])
                else:
                    nc.scalar.activation(osb[:], oT[:], COPYF)
                nc.sync.dma_start(out[ii, win2], osb[:])

        for i in range(HPC):
            qt, kt, vt = qts[i], kts[i], vts[i]

            for win in range(NWIN - 1, -1, -1):  # largest window first
                qlo = 512 * win
                qhi = qlo + 512
                if win == 2 and i + 1 < HPC:
                    alloc_head(i + 1)  # prefetch next head's inputs

                oT = po_pool.tile([65, 512], F32, tag="oT", name=f"oT{i}_{win}")
                nlast = 4 * win + 3

                for p in range((nlast + 1) // 4):
                    group = []
                    for n in range(4 * p, 4 * p + 4):
                        q0 = max(128 * n, qlo)
                        w = qhi - q0
                        ps = ps_pool.tile(
                            [128, 512], F32, tag="ps", name=f"ps{i}_{win}_{n}"
                        )
                        group.append((n, q0, w, ps))
                    # adjacent emission -> the 64-row halves run concurrently
                    for n, q0, w, ps in group:
                        if n >= 4 * win:
                            # diag group: pair (512,384) and (256,128) would
                            # load half A with 768 cols vs 512; assigning
                            # {j0,j3}->A, {j1,j2}->B balances both at 640.
                            half = 0 if (n - 4 * win) in (0, 3) else 1
                        else:
                            half = n & 1
                        pr = slice(64 * half, 64 * half + 64)
                        nc.tensor.matmul(
                            ps[:, 0:w],
                            lhsT=kt[pr, 128 * n : 128 * n + 128],
                            rhs=qt[pr, q0:qhi],
                            start=True,
                            stop=True,
                            tile_position=(64 * half, 0),
                        )
                    for n, q0, w, ps in group:
                        a_sb = a_pool.tile(
                            [128, 512], BF16, tag="a", name=f"a{i}_{win}_{n}"
                        )
                        diag = n >= 4 * win
                        if diag:
                            if n == 4 * win:
                                # largest diag chunk: exact exp over the whole
                                # width on ScalarE (mask region included), then
                                # the idle GpSimd zeroes the s>q triangle of
                                # the 128-col diagonal block in SBUF.
                                nc.scalar.activation(
                                    a_sb[:, 0:w], ps[:, 0:w], EXP, scale=SCALE
                                )
                                nc.gpsimd.affine_select(
                                    out=a_sb[:, 0:128],
                                    in_=a_sb[:, 0:128],
                                    compare_op=mybir.AluOpType.is_ge,
                                    fill=0.0,
                                    base=0,
                                    pattern=[[1, 128]],
                                    channel_multiplier=-1,
                                )
                            else:
                                nc.vector.scalar_tensor_tensor(
                                    out=a_sb[:, 0:w].bitcast(I16),
                                    in0=ps[:, 0:w],
                                    scalar=C1,
                                    in1=tribias[:, 0:w],
                                    op0=MULT,
                                    op1=ADD,
                                )
                        elif n % 2 == 0:
                            nc.vector.tensor_scalar(
                                a_sb[:, 0:w].bitcast(I16),
                                ps[:, 0:w],
                                C1,
                                BIAS,
                                MULT,
                                ADD,
                            )
                        else:
                            nc.scalar.activation(
                                a_sb[:, 0:w], ps[:, 0:w], EXP, scale=SCALE
                            )
                        pend.append((n, q0, w, a_sb, oT, qlo, nlast, vt, i, win))

                    while len(pend) > LAG:
                        emit_av(pend.pop(0))
        for u in pend:
            emit_av(u)


_CACHED = None


def _build():
    global _CACHED
    if _CACHED is not None:
        return _CACHED
    nc = bacc.Bacc("TRN2", target_bir_lowering=False, debug=False)
    qT = nc.dram_tensor("qT", [HPC, 128, L], BF16, kind="ExternalInput").ap()
    kT = nc.dram_tensor("kT", [HPC, 128, L], BF16, kind="ExternalInput").ap()
    v1 = nc.dram_tensor("v1", [HPC, 128, NCH * 65], BF16, kind="ExternalInput").ap()
    out = nc.dram_tensor("out", [HPC, NWIN, 65, 512], F32, kind="ExternalOutput").ap()
    with tile.TileContext(nc) as tc:
        _body(tc, qT, kT, v1, out)
    nc.compile()
    _CACHED = nc
    return nc


def _prep_in_maps(queries, keys, values, tau, delta):
    """Shard + relayout the full inputs into 8 per-core input dicts."""
    queries = np.asarray(queries, dtype=np.float32)
    keys = np.asarray(keys, dtype=np.float32)
    values = np.asarray(values, dtype=np.float32)
    tau = np.asarray(tau, dtype=np.float32)
    delta = np.asarray(delta, dtype=np.float32)

    ktau = keys * tau[:, 0][:, None, None, None]
    D = np.exp(delta / 8.0)  # [B, L]
    vD = values * D[:, :, None, None]

    in_maps = []
    for core in range(NCORES):
        qTs = np.empty((HPC, 128, L), ml_dtypes.bfloat16)
        kTs = np.empty((HPC, 128, L), ml_dtypes.bfloat16)
        v1s = np.empty((HPC, 128, NCH * 65), ml_dtypes.bfloat16)
        for slot in range(HPC):
            g = core * HPC + slot
            b, h = divmod(g, H)
            qtv = queries[b, :, h, :].T.astype(ml_dtypes.bfloat16)
            ktv = ktau[b, :, h, :].T.astype(ml_dtypes.bfloat16)
            # duplicate across both partition halves for PE row tiling
            qTs[slot, 0:64] = qtv
            qTs[slot, 64:128] = qtv
            kTs[slot, 0:64] = ktv
            kTs[slot, 64:128] = ktv
            v = vD[b, :, h, :].reshape(NCH, 128, E).transpose(1, 0, 2)
            dd = D[b].reshape(NCH, 128).T[:, :, None]
            vv = np.concatenate([v, dd], axis=2)
            v1s[slot] = vv.reshape(128, NCH * 65).astype(ml_dtypes.bfloat16)
        in_maps.append({"qT": qTs, "kT": kTs, "v1": v1s})
    return in_maps


def _assemble(results):
    O = np.empty((B, L, H, E), np.float32)
    allo = np.stack([results[c]["out"] for c in range(NCORES)])
    # [8, HPC, NWIN, 65, 512]
    num = allo[:, :, :, 0:64, :]
    den = allo[:, :, :, 64:65, :]
    ot = num / den
    # [core, slot, win, e, col] -> [core, slot, L, E]
    ot = ot.transpose(0, 1, 2, 4, 3).reshape(NCORES, HPC, L, E)
    for core in range(NCORES):
        for slot in range(HPC):
            g = core * HPC + slot
            b, h = divmod(g, H)
            O[b, :, h, :] = ot[core, slot]
    return O


def run(inputs, trace=False, **kwargs):
    from concourse import bass_utils

    nc = _build()
    in_maps = _prep_in_maps(**inputs)
    res = bass_utils.run_bass_kernel_spmd(
        nc, in_maps, core_ids=list(range(NCORES)), trace=trace, **kwargs
    )
    return _assemble(res.results), res


def kernel(**inputs):
    return run(inputs, trace=False)[0]
